# revision 1
# baseline (speedup 1.0000x reference)
"""GCN (3-layer + mean-pool head) on 8 Trainium2 cores.

Architecture (per core c, nodes sharded contiguously):
  z0 = S x          gather x rows per edge (dst on this core) + indicator matmuls in PSUM
  h1 = relu(z0 W1 + b1)   dense per 128-node chunk, feature-major
  AllGather h1 (node-major table) across cores
  z1 = S h1         same machinery, table = h1full
  h2 = relu(z1 W2 + b2)
  q  = h2 @ (W3 linW)     [nodes, 1]
  out[g] = sum_e coef[e] q[src[e]] + c   via per-chunk C-matrix matmuls, C host-built
Host sums the 8 partial outs.

Sparse aggregation: edges grouped by (dst chunk of 128, window j of 32, src-table half),
padded to 128-edge blocks; block matmul  z[:F, 32j:32j+32] += msg_blk.T @ ind_blk
with msg_blk = gathered [128 edges, F], ind_blk = one-hot*norm [128, 32] (host-built).
Block counts equalized across cores so all 8 cores share one NEFF.
"""

from dataclasses import dataclass
import numpy as np

import concourse.bass as bass
import concourse.bacc as bacc
import concourse.mybir as mybir
import concourse.tile as tile
from concourse.masks import make_identity

BLK = 128  # edges per block
W = 32  # dst window width
NW = 4  # windows per chunk


@dataclass
class Cfg:
    N: int = 50000
    E: int = 1000000
    G: int = 128
    FIN: int = 64
    H: int = 128
    H2: int = 256
    NC: int = 8
    CG: int = 4  # chunks per gather group
    SPLIT: int = 32768  # int16 table split

    @property
    def NPC(self):
        assert self.N % self.NC == 0
        return self.N // self.NC

    @property
    def CH(self):
        return (self.NPC + 127) // 128

    @property
    def PADN(self):
        return self.CH * 128

    @property
    def NG(self):
        return (self.CH + self.CG - 1) // self.CG


def _ceil_div(a, b):
    return -(-a // b)


class LayerStruct:
    """Static (cross-core shared) block structure + per-core data for one sparse layer.

    Blocks per (chunk k, half): per window j, full 128-edge blocks of width 32;
    all remainders concatenated into chunk-wide tail blocks (width 128). Block
    counts are equalized across cores (max) so all 8 cores share one NEFF.
    """

    def __init__(self, cfg: Cfg, rows, src_unused, dst, norm, n_table_rows):
        NC, CH, NPC, CG, SPLIT = cfg.NC, cfg.CH, cfg.NPC, cfg.CG, cfg.SPLIT
        core = dst // NPC
        l = dst - core * NPC
        k = l >> 7
        j = (l >> 5) & 3
        w32 = l & 31
        w128 = l & 127
        half = (rows >= SPLIT).astype(np.int64)
        self.n_lo_rows = min(SPLIT, n_table_rows)
        self.n_hi_rows = max(0, n_table_rows - SPLIT)

        key = (((core * CH + k) * 2 + half) * NW + j)
        counts = np.bincount(key, minlength=NC * CH * 2 * NW).reshape(NC, CH, 2, NW)
        Bfull = (counts // BLK).max(axis=0)  # [CH, 2, NW]
        leftover = counts - np.minimum(counts, Bfull[None] * BLK)  # per-core leftovers
        tail_cnt = leftover.sum(axis=3)  # [NC, CH, 2]
        Btail = _ceil_div(tail_cnt, BLK).max(axis=0)  # [CH, 2]
        self.Bfull, self.Btail = Bfull, Btail

        # canonical enumeration: groups -> halves -> chunks -> (full j-blocks, tails)
        # block id and indicator column offset assigned in this order
        full_base = np.zeros((CH, 2, NW), dtype=np.int64)
        tail_base = np.zeros((CH, 2), dtype=np.int64)
        ind_off = {}
        cur = 0
        icol = 0
        self.groups = []
        self.chunk_blocks = [None] * CH
        for g in range(cfg.NG):
            ks = range(g * CG, min((g + 1) * CG, CH))
            first_blk, first_icol = cur, icol
            half_cnt = [0, 0]
            for h in (0, 1):
                for kk in ks:
                    for jj in range(NW):
                        full_base[kk, h, jj] = cur
                        for b in range(Bfull[kk, h, jj]):
                            ind_off[cur] = icol
                            icol += W
                            cur += 1
                    tail_base[kk, h] = cur
                    for b in range(Btail[kk, h]):
                        ind_off[cur] = icol
                        icol += BLK
                        cur += 1
                half_cnt[h] = (
                    int(Bfull[list(ks), h].sum()) + int(Btail[list(ks), h].sum())
                )
            self.groups.append(
                dict(
                    chunks=list(ks),
                    first_blk=first_blk,
                    first_icol=first_icol,
                    lo_cnt=half_cnt[0],
                    hi_cnt=half_cnt[1],
                    ind_cols=icol - first_icol,
                )
            )
            # per-chunk emission metadata: (half, call_slot, rel_icol, width, out_off)
            for kk in ks:
                bl = []
                for h in (0, 1):
                    for jj in range(NW):
                        for b in range(Bfull[kk, h, jj]):
                            bg = int(full_base[kk, h, jj]) + b
                            cs = bg - first_blk - (half_cnt[0] if h else 0)
                            bl.append((h, cs, ind_off[bg] - first_icol, W, jj * W))
                    for b in range(Btail[kk, h]):
                        bg = int(tail_base[kk, h]) + b
                        cs = bg - first_blk - (half_cnt[0] if h else 0)
                        bl.append((h, cs, ind_off[bg] - first_icol, BLK, 0))
                self.chunk_blocks[kk] = bl
        self.TOT = cur
        self.IND_COLS = icol

        # map block id -> call slot base for slot computation
        # per-core slot assignment (vectorized)
        order = np.lexsort((j, key))
        sk = key[order]
        newgrp = np.ones(len(sk), dtype=bool)
        newgrp[1:] = sk[1:] != sk[:-1]
        starts = np.flatnonzero(newgrp)
        lengths = np.diff(np.append(starts, len(sk)))
        rank_sorted = np.arange(len(sk)) - np.repeat(starts, lengths)
        rank = np.empty(len(sk), dtype=np.int64)
        rank[order] = rank_sorted  # rank within (core,k,half,j)

        capacity = Bfull[k, half, j] * BLK  # [E]
        is_full = rank < capacity
        # full-block slot
        blk_full = full_base[k, half, j] + rank // BLK
        # tail assignment: leftover rank within (core,k,half) = prefix-leftovers
        lo_pref = np.cumsum(leftover, axis=3) - leftover  # exclusive prefix by j
        tail_rank = lo_pref[core, k, half, j] + (rank - capacity)
        blk_tail = tail_base[k, half] + tail_rank // BLK
        blk = np.where(is_full, blk_full, blk_tail)
        slot = np.where(is_full, rank % BLK, tail_rank % BLK)
        wcol = np.where(is_full, w32, w128)
        # indicator column: ind_off lookup per block id
        ind_off_arr = np.zeros(self.TOT, dtype=np.int64)
        for bg, col in ind_off.items():
            ind_off_arr[bg] = col
        s_global = blk * BLK + slot

        self.per_core = []
        for c in range(NC):
            m = core == c
            ncols = self.TOT * BLK // 16
            idx16 = np.zeros((16, ncols), dtype=np.int16)
            sg = s_global[m]
            vals = (rows[m] - half[m] * SPLIT).astype(np.int16)
            idx16[sg % 16, sg // 16] = vals
            idx_arr = np.tile(idx16, (8, 1))
            ind_arr = np.zeros((BLK, self.IND_COLS), dtype=np.float32)
            ind_arr[slot[m], ind_off_arr[blk[m]] + wcol[m]] = norm[m]
            self.per_core.append((idx_arr, ind_arr))


def preprocess(cfg: Cfg, inputs):
    x = np.asarray(inputs["x"], dtype=np.float32)
    ei = np.asarray(inputs["edge_index"], dtype=np.int64)
    batch = np.asarray(inputs["batch"], dtype=np.int64)
    W1 = np.asarray(inputs["W1"], np.float32)
    b1 = np.asarray(inputs["b1"], np.float32)
    W2 = np.asarray(inputs["W2"], np.float32)
    b2 = np.asarray(inputs["b2"], np.float32)
    W3 = np.asarray(inputs["W3"], np.float32)
    b3 = np.asarray(inputs["b3"], np.float32)
    linW = np.asarray(inputs["linW"], np.float32)
    linb = np.asarray(inputs["linb"], np.float32)

    N, NC, NPC, PADN, CH, G = cfg.N, cfg.NC, cfg.NPC, cfg.PADN, cfg.CH, cfg.G
    src = np.concatenate([ei[0], np.arange(N, dtype=np.int64)])
    dst = np.concatenate([ei[1], np.arange(N, dtype=np.int64)])
    deg = np.bincount(dst, minlength=N).astype(np.float32)
    dinv = 1.0 / np.sqrt(deg)
    norm = (dinv[src] * dinv[dst]).astype(np.float32)

    # L1: table = x, row = src
    L1 = LayerStruct(cfg, src, src, dst, norm, n_table_rows=N)
    # L2: table = h1full [NC*PADN, H], row = r(n)
    r_of = (src // NPC) * PADN + (src % NPC)
    L2 = LayerStruct(cfg, r_of, src, dst, norm, n_table_rows=NC * PADN)

    # L3: C matrices [NC, CH*128, G]
    cnt = np.maximum(np.bincount(batch, minlength=G), 1).astype(np.float32)
    coef = norm / cnt[batch[dst]]
    c_src = src // NPC
    kk = (src % NPC) >> 7
    ll = (src % NPC) & 127
    gg = batch[dst]
    flat = ((c_src * CH + kk) * 128 + ll) * G + gg
    C = np.bincount(flat, weights=coef.astype(np.float64), minlength=NC * CH * 128 * G)
    C = C.reshape(NC, CH * 128, G).astype(np.float32)

    w3 = (W3 @ linW).astype(np.float32)  # [H2, 1]
    c_const = float(b3 @ linW[:, 0] + linb[0])
    empty = np.bincount(batch, minlength=G) == 0

    H, H2 = cfg.H, cfg.H2
    in_maps = []
    for c in range(NC):
        idx1, ind1 = L1.per_core[c]
        idx2, ind2 = L2.per_core[c]
        in_maps.append(
            {
                "x": x,
                "W1": W1,
                "b1": b1.reshape(H, 1),
                "W2": W2,
                "b2": b2.reshape(2, H).T.copy(),
                "w3": w3.reshape(2, H).T.copy(),
                "idx1": idx1,
                "ind1": ind1,
                "idx2": idx2,
                "ind2": ind2,
                "C": C[c],
            }
        )
    host = dict(c_const=c_const, empty=empty, linb=float(linb[0]))
    return L1, L2, in_maps, host


def build_module(cfg: Cfg, L1: LayerStruct, L2: LayerStruct, stop_after: str = 'full', single_core: bool = False, probe: str = ''):
    N, NC, PADN, CH, G = cfg.N, cfg.NC, cfg.PADN, cfg.CH, cfg.G
    FIN, H, H2 = cfg.FIN, cfg.H, cfg.H2
    f32 = mybir.dt.float32

    nc = bacc.Bacc("TRN2", debug=False, num_devices=1 if single_core else NC)
    x_t = nc.dram_tensor("x", [N, FIN], f32, kind="ExternalInput")
    W1_t = nc.dram_tensor("W1", [FIN, H], f32, kind="ExternalInput")
    b1_t = nc.dram_tensor("b1", [H, 1], f32, kind="ExternalInput")
    W2_t = nc.dram_tensor("W2", [H, H2], f32, kind="ExternalInput")
    b2_t = nc.dram_tensor("b2", [H, 2], f32, kind="ExternalInput")
    w3_t = nc.dram_tensor("w3", [H, 2], f32, kind="ExternalInput")
    idx1_t = nc.dram_tensor("idx1", [128, L1.TOT * 8], mybir.dt.int16, kind="ExternalInput")
    ind1_t = nc.dram_tensor("ind1", [128, L1.IND_COLS], f32, kind="ExternalInput")
    idx2_t = nc.dram_tensor("idx2", [128, L2.TOT * 8], mybir.dt.int16, kind="ExternalInput")
    ind2_t = nc.dram_tensor("ind2", [128, L2.IND_COLS], f32, kind="ExternalInput")
    C_t = nc.dram_tensor("C", [CH * 128, G], f32, kind="ExternalInput")
    if stop_after == 'full':
        out_t = nc.dram_tensor("out", [G, 1], f32, kind="ExternalOutput")
    else:
        dbg_t = nc.dram_tensor("dbg", [PADN, H], f32, kind="ExternalOutput")

    h1sh = nc.dram_tensor("h1sh", [PADN, H], f32)
    h1full = nc.dram_tensor("h1full", [NC * PADN, H], f32, addr_space="Shared")

    with tile.TileContext(nc) as tc:
        with (
            tc.tile_pool(name="const", bufs=1) as cpool,
            tc.tile_pool(name="idx", bufs=2) as idxp,
            tc.tile_pool(name="msg", bufs=2) as msgp,
            tc.tile_pool(name="indp", bufs=2) as indp,
            tc.tile_pool(name="sb", bufs=2) as sbp,
            tc.tile_pool(name="qpool", bufs=1) as qpool,
            tc.tile_pool(name="zps", bufs=2, space="PSUM") as zpsp,
            tc.tile_pool(name="hps", bufs=2, space="PSUM") as hpsp,
            tc.tile_pool(name="tps", bufs=1, space="PSUM") as tpsp,
            tc.tile_pool(name="qps", bufs=1, space="PSUM") as qpsp,
            tc.tile_pool(name="pps", bufs=1, space="PSUM") as ppsp,
            tc.tile_pool(name="scr", bufs=1, space="PSUM") as scrp,
        ):
            zero_sb = cpool.tile([128, 128], f32)
            nc.vector.memset(zero_sb[:], 0.0)
            ident = cpool.tile([128, 128], f32)
            make_identity(nc, ident[:])
            W1_sb = cpool.tile([FIN, H], f32)
            nc.sync.dma_start(out=W1_sb[:], in_=W1_t[:, :])
            b1_sb = cpool.tile([H, 1], f32)
            nc.sync.dma_start(out=b1_sb[:], in_=b1_t[:, :])
            W2_sb = cpool.tile([H, H2], f32)
            nc.sync.dma_start(out=W2_sb[:], in_=W2_t[:, :])
            b2_sb = cpool.tile([H, 2], f32)
            nc.sync.dma_start(out=b2_sb[:], in_=b2_t[:, :])
            w3_sb = cpool.tile([H, 2], f32)
            nc.sync.dma_start(out=w3_sb[:], in_=w3_t[:, :])
            scr_ps = scrp.tile([1, 1], f32, space="PSUM")
            q_sb = qpool.tile([128, CH], f32)
            pool_ps = ppsp.tile([G, 1], f32, space="PSUM")

            def absorb(dep_ap):
                # dummy matmul so each fresh cross-engine sem lands on its own
                # PE instruction (walrus allows ~1 sync wait per Matmult)
                kdim = dep_ap.shape[0]
                nc.tensor.matmul(
                    scr_ps[:], lhsT=zero_sb[:kdim, :1], rhs=dep_ap, start=True, stop=True
                )

            absorb(zero_sb[:, :1])
            for cst in (ident, W1_sb, b1_sb, W2_sb, b2_sb, w3_sb):
                absorb(cst[:, :1])
            # ACT-engine absorbers: the activation instruction also allows only
            # one sync wait, so its bias-DMA sems must be pre-observed by ACT
            act_scr = cpool.tile([H, 3], f32)
            nc.scalar.copy(act_scr[:, 0:1], b1_sb[:, :1])
            nc.scalar.copy(act_scr[:, 1:2], b2_sb[:, 0:1])
            nc.scalar.copy(act_scr[:, 2:3], b2_sb[:, 1:2])

            def sparse_layer(LS: LayerStruct, F, idx_t, ind_t, lo_ap, hi_ap, consume_chunk):
                """Emit gather+indicator-matmul aggregation; call consume_chunk(k, z_sb [F,128])."""
                for g_i, g in enumerate(LS.groups):
                    fb = g["first_blk"]
                    nlo, nhi = g["lo_cnt"], g["hi_cnt"]
                    msg_tiles = {}
                    for h, cnt_, table_ap in ((0, nlo, lo_ap), (1, nhi, hi_ap)):
                        if cnt_ == 0:
                            continue
                        if 'nogather' in probe:
                            msg_nog = msgp.tile([128, cnt_ * F], f32, tag=f"msg_{h}")
                            nc.vector.memset(msg_nog[:, :1], 0.0)
                            msg_tiles[h] = msg_nog
                            continue
                        nidx = cnt_ * BLK
                        col0 = (fb + (nlo if h else 0)) * 8
                        idx_sb = idxp.tile([128, nidx // 16], mybir.dt.int16, tag=f"idx{h}")  # shared across layers
                        nc.sync.dma_start(
                            out=idx_sb[:], in_=idx_t[:, col0 : col0 + nidx // 16]
                        )
                        msg = msgp.tile([128, cnt_ * F], f32, tag=f"msg_{h}")
                        nc.gpsimd.dma_gather(
                            msg[:].rearrange("p (b f) -> p b f", b=cnt_),
                            table_ap,
                            idx_sb[:],
                            num_idxs=nidx,
                            num_idxs_reg=nidx,
                            elem_size=F,
                            single_packet=False,
                        )
                        msg_tiles[h] = msg
                    ic0, icn = g["first_icol"], g["ind_cols"]
                    ind_sb = indp.tile([128, icn], f32, tag="ind")
                    if 'noind' not in probe:
                        nc.sync.dma_start(out=ind_sb[:], in_=ind_t[:, ic0 : ic0 + icn])
                    else:
                        nc.vector.memset(ind_sb[:, :1], 0.0)
                    # wait-absorbers: consume each fresh DMA sem on its own PE inst
                    # (walrus allows only ~1 sync wait per Matmult/LDWEIGHTS)
                    for dep in (*msg_tiles.values(), ind_sb):
                        if 'noabsorb' in probe:
                            break
                        absorb(dep[:, :1])
                    for kk in g["chunks"]:
                        blocks = LS.chunk_blocks[kk]
                        zps = zpsp.tile([128, 128], f32, space="PSUM", tag="z")
                        nc.tensor.matmul(
                            zps[:F, :],
                            lhsT=zero_sb[:, :F],
                            rhs=zero_sb[:, :],
                            start=True,
                            stop=False,
                        )
                        for bi, (h, cs, ric, width, ooff) in enumerate(blocks):
                            if 'noblocks' in probe:
                                break
                            last = bi == len(blocks) - 1
                            msg = msg_tiles[h]
                            nc.tensor.matmul(
                                zps[:F, ooff : ooff + width],
                                lhsT=msg[:, cs * F : (cs + 1) * F],
                                rhs=ind_sb[:, ric : ric + width],
                                start=False,
                                stop=last,
                            )
                        z_sb = sbp.tile([F, 128], f32, tag="z_sb")
                        nc.vector.tensor_copy(out=z_sb[:], in_=zps[:F, :])
                        consume_chunk(kk, z_sb)

            # ---- Layer 1 ----
            def l1_chunk(kk, z_sb):
                absorb(z_sb[:, :1])
                hps = hpsp.tile([H, 128], f32, space="PSUM", tag="h")
                nc.tensor.matmul(hps[:], lhsT=W1_sb[:], rhs=z_sb[:FIN, :], start=True, stop=True)
                h1T = sbp.tile([H, 128], f32, tag="h1T")
                nc.scalar.activation(
                    h1T[:], hps[:], mybir.ActivationFunctionType.Relu, bias=b1_sb[:, :]
                )
                absorb(h1T[:, :1])
                tps = tpsp.tile([128, H], f32, space="PSUM", tag="t")
                nc.tensor.transpose(out=tps[:], in_=h1T[:], identity=ident[:])
                h1n = sbp.tile([128, H], f32, tag="h1n")
                nc.vector.tensor_copy(out=h1n[:], in_=tps[:])
                nc.sync.dma_start(out=h1sh[kk * 128 : (kk + 1) * 128, :], in_=h1n[:])

            sparse_layer(
                L1, FIN, idx1_t, ind1_t,
                x_t[0 : L1.n_lo_rows, :],
                x_t[L1.n_lo_rows : N, :] if L1.n_hi_rows else x_t[0:1, :],
                l1_chunk,
            )

            if stop_after == 'l1':
                dsb = sbp.tile([128, H], f32, tag="dbg")
                for kk in range(CH):
                    nc.sync.dma_start(out=dsb[:], in_=h1sh[kk * 128 : (kk + 1) * 128, :])
                    nc.sync.dma_start(out=dbg_t[kk * 128 : (kk + 1) * 128, :], in_=dsb[:])
                nc.compile()
                return nc

            # ---- AllGather h1 ----
            if single_core:
                # timing-representative stand-in (no collectives in TimelineSim)
                nc.sync.dma_start(out=h1full[0:PADN, :], in_=h1sh[:, :])
            else:
                nc.gpsimd.collective_compute(
                    "AllGather",
                    mybir.AluOpType.bypass,
                    replica_groups=[list(range(NC))],
                    ins=[h1sh[:, :]],
                    outs=[h1full[:, :]],
                )

            if stop_after == 'ag':
                dsb = sbp.tile([128, H], f32, tag="dbg")
                for kk in range(CH):
                    nc.sync.dma_start(out=dsb[:], in_=h1full[kk * 128 : (kk + 1) * 128, :])
                    nc.sync.dma_start(out=dbg_t[kk * 128 : (kk + 1) * 128, :], in_=dsb[:])
                nc.compile()
                return nc

            # ---- Layer 2 + head ----
            def l2_chunk(kk, z_sb):
                absorb(z_sb[:, :1])
                h2T_halves = []
                for half_i in range(2):
                    hps = hpsp.tile([H, 128], f32, space="PSUM", tag="h")
                    nc.tensor.matmul(
                        hps[:],
                        lhsT=W2_sb[:, half_i * H : (half_i + 1) * H],
                        rhs=z_sb[:],
                        start=True,
                        stop=True,
                    )
                    h2T = sbp.tile([H, 128], f32, tag=f"h2T{half_i}")
                    nc.scalar.activation(
                        h2T[:],
                        hps[:],
                        mybir.ActivationFunctionType.Relu,
                        bias=b2_sb[:, half_i : half_i + 1],
                    )
                    h2T_halves.append(h2T)
                absorb(h2T_halves[0][:, :1])
                absorb(h2T_halves[1][:, :1])
                qps = qpsp.tile([128, 1], f32, space="PSUM", tag="q")
                for half_i in range(2):
                    nc.tensor.matmul(
                        qps[:],
                        lhsT=h2T_halves[half_i][:],
                        rhs=w3_sb[:, half_i : half_i + 1],
                        start=half_i == 0,
                        stop=half_i == 1,
                    )
                nc.vector.tensor_copy(out=q_sb[:, kk : kk + 1], in_=qps[:])
                Cs = sbp.tile([128, G], f32, tag="Cs")
                nc.sync.dma_start(out=Cs[:], in_=C_t[kk * 128 : (kk + 1) * 128, :])
                absorb(Cs[:, :1])
                nc.tensor.matmul(
                    pool_ps[:],
                    lhsT=Cs[:],
                    rhs=q_sb[:, kk : kk + 1],
                    start=kk == 0,
                    stop=kk == CH - 1,
                )

            sparse_layer(
                L2, H, idx2_t, ind2_t,
                h1full[0 : L2.n_lo_rows, :],
                h1full[L2.n_lo_rows : NC * PADN, :] if L2.n_hi_rows else h1full[0:1, :],
                l2_chunk,
            )

            pool_sb = sbp.tile([G, 1], f32, tag="pool")
            nc.vector.tensor_copy(out=pool_sb[:], in_=pool_ps[:])
            nc.sync.dma_start(out=out_t[:, :], in_=pool_sb[:])

    nc.compile()
    return nc


def postprocess(cfg: Cfg, results, host):
    out = np.zeros((cfg.G, 1), dtype=np.float64)
    for r in results:
        out += r["out"].astype(np.float64)
    out += host["c_const"]
    out[host["empty"], 0] = host["linb"]
    return out.astype(np.float32)


# ---------------------------------------------------------------------------
# Harness entry point: full inputs in, full output out. Shards across the 8
# NeuronCores internally (dst-sharded edges, replicated weights, AllGather h1).
# ---------------------------------------------------------------------------
from concourse import bass_utils as _bass_utils


def kernel(**inputs) -> np.ndarray:
    cfg = Cfg()
    L1, L2, in_maps, host = preprocess(cfg, inputs)
    nc = build_module(cfg, L1, L2)
    res = _bass_utils.run_bass_kernel_spmd(nc, in_maps, core_ids=list(range(cfg.NC)))
    return postprocess(cfg, res.results, host)



# revision 2
# speedup vs baseline: 1.0967x; 1.0967x over previous
"""GCN (3-layer + mean-pool head) on 8 Trainium2 cores — v3 = v2 + bf16 L2 path.

bf16: h1 table (halves L2 gather traffic + h1 AllGather), L2 msg/ind matmuls
(4x PE stream rate vs fp32), C matrix + q (halves head DMA). L1 stays fp32.
"""

_V2_DOC = """GCN (3-layer + mean-pool head) on 8 Trainium2 cores — v2, slim inputs.

Differences from v1:
  - ind matrices built ON DEVICE from per-slot (wcol, norm) arrays via
    iota-ramp is_equal + multiply (upload 1.2MB/layer instead of 30MB).
  - idx uploaded un-tiled [16, TOT*8] and replicated to 128 partitions on
    device (0.3MB instead of 2.4MB per layer).
  - x uploaded sharded [NPC, FIN] per core and AllGathered on device
    (1.6MB instead of 12.8MB per core).
Per-group block enumeration: gather order per half = fulls then tails;
ind columns: fulls region (width 32 each) then tails region (width 128).
"""  # noqa: E501

from dataclasses import dataclass
import numpy as np

import concourse.bass as bass
import concourse.bacc as bacc
import concourse.mybir as mybir
import concourse.tile as tile
from concourse.masks import make_identity

BLK = 128
W = 32
NW = 4


@dataclass
class Cfg:
    N: int = 50000
    E: int = 1000000
    G: int = 128
    FIN: int = 64
    H: int = 128
    H2: int = 256
    NC: int = 8
    CG: int = 4
    SPLIT: int = 32768

    @property
    def NPC(self):
        assert self.N % self.NC == 0
        return self.N // self.NC

    @property
    def CH(self):
        return (self.NPC + 127) // 128

    @property
    def PADN(self):
        return self.CH * 128

    @property
    def NG(self):
        return (self.CH + self.CG - 1) // self.CG


def _ceil_div(a, b):
    return -(-a // b)


class LayerStruct:
    """Block structure shared across cores + per-core compact arrays.

    Per group g (CG chunks):
      gather order: half h: [fulls(k asc, j asc, b), tails(k asc, b)] -> cs
      ind columns:  fulls region [fulls h0 ++ fulls h1] (width 32 each),
                    tails region [tails h0 ++ tails h1] (width 128 each)
      idx16 columns: group base gcol0 = first_blk*8; h0 blocks then h1 blocks
        in gather order, 8 int16 cols per block.
    Per-core arrays:
      idx16 [16, TOT*8]   wrapped gather indices (block-major in gather order)
      wcol  [128, TOT_ind] f32 window col per slot (ind order: per group fulls
                           then tails, concatenated over groups)
      nval  [128, TOT_ind] f32 norm per slot (0 padding)
    """

    def __init__(self, cfg: Cfg, rows, dst, norm, n_table_rows):
        NC, CH, NPC, CG, SPLIT = cfg.NC, cfg.CH, cfg.NPC, cfg.CG, cfg.SPLIT
        core = dst // NPC
        l = dst - core * NPC
        k = l >> 7
        j = (l >> 5) & 3
        w32 = l & 31
        w128 = l & 127
        half = (rows >= SPLIT).astype(np.int64)
        self.n_lo_rows = min(SPLIT, n_table_rows)
        self.n_hi_rows = max(0, n_table_rows - SPLIT)

        key = (((core * CH + k) * 2 + half) * NW + j)
        counts = np.bincount(key, minlength=NC * CH * 2 * NW).reshape(NC, CH, 2, NW)
        Bfull = (counts // BLK).max(axis=0)  # [CH, 2, NW]
        leftover = counts - np.minimum(counts, Bfull[None] * BLK)
        tail_cnt = leftover.sum(axis=3)  # [NC, CH, 2]
        Btail = _ceil_div(tail_cnt, BLK).max(axis=0)  # [CH, 2]
        self.Bfull, self.Btail = Bfull, Btail

        # --- enumerate blocks ---
        # per (k,h,j): gather cs base; per (k,h): tail cs base
        # per block: ind column offset (fulls then tails region per group)
        full_cs = np.zeros((CH, 2, NW), dtype=np.int64)  # cs of first full blk
        tail_cs = np.zeros((CH, 2), dtype=np.int64)
        full_sg = np.zeros((CH, 2, NW), dtype=np.int64)  # global gather slot base
        tail_sg = np.zeros((CH, 2), dtype=np.int64)
        full_ic = np.zeros((CH, 2, NW), dtype=np.int64)  # ind col offset (global)
        tail_ic = np.zeros((CH, 2), dtype=np.int64)
        # ind-order column index (into wcol/nval [*, TOT_ind]) per block
        full_bc = np.zeros((CH, 2, NW), dtype=np.int64)
        tail_bc = np.zeros((CH, 2), dtype=np.int64)

        self.groups = []
        self.chunk_blocks = [None] * CH  # list of (h, cs, ric_kind, roff, width, ooff)
        cur_blk = 0  # global block counter (gather order, h-grouped per group)
        cur_ic = 0  # global ind col counter
        cur_bc = 0  # global ind-order block col counter
        for g in range(cfg.NG):
            ks = list(range(g * CG, min((g + 1) * CG, CH)))
            first_blk = cur_blk
            # gather order per half
            half_cnt = [0, 0]
            for h in (0, 1):
                cs = 0
                for kk in ks:
                    for jj in range(NW):
                        full_cs[kk, h, jj] = cs
                        cs += Bfull[kk, h, jj]
                for kk in ks:
                    tail_cs[kk, h] = cs
                    cs += Btail[kk, h]
                half_cnt[h] = cs
            nlo, nhi = half_cnt
            for h in (0, 1):
                base = first_blk + (nlo if h else 0)
                for kk in ks:
                    for jj in range(NW):
                        full_sg[kk, h, jj] = (base + full_cs[kk, h, jj]) * BLK
                    tail_sg[kk, h] = (base + tail_cs[kk, h]) * BLK
            # ind columns: fulls h0 ++ fulls h1, then tails h0 ++ tails h1
            first_ic = cur_ic
            first_bc = cur_bc
            nf = 0
            for h in (0, 1):
                for kk in ks:
                    for jj in range(NW):
                        full_ic[kk, h, jj] = cur_ic
                        full_bc[kk, h, jj] = cur_bc
                        cur_ic += Bfull[kk, h, jj] * W
                        cur_bc += Bfull[kk, h, jj]
                        nf += Bfull[kk, h, jj]
            ic_tail0 = cur_ic
            bc_tail0 = cur_bc
            nt = 0
            for h in (0, 1):
                for kk in ks:
                    tail_ic[kk, h] = cur_ic
                    tail_bc[kk, h] = cur_bc
                    cur_ic += Btail[kk, h] * BLK
                    cur_bc += Btail[kk, h]
                    nt += Btail[kk, h]
            cur_blk += nlo + nhi
            self.groups.append(
                dict(
                    chunks=ks,
                    first_blk=first_blk,
                    lo_cnt=nlo,
                    hi_cnt=nhi,
                    nf=nf,
                    nt=nt,
                    first_ic=first_ic,  # fulls ind region start (global col)
                    tail_ic0=ic_tail0,  # tails ind region start
                    first_bc=first_bc,  # fulls block-col start in wcol/nval
                    tail_bc0=bc_tail0,
                )
            )
            # per-chunk emission metadata
            for kk in ks:
                bl = []
                for h in (0, 1):
                    for jj in range(NW):
                        for b in range(Bfull[kk, h, jj]):
                            cs = full_cs[kk, h, jj] + b
                            ric = full_ic[kk, h, jj] + b * W - first_ic
                            bl.append((h, cs, "full", ric, W, jj * W))
                    for b in range(Btail[kk, h]):
                        cs = tail_cs[kk, h] + b
                        ric = tail_ic[kk, h] + b * BLK - ic_tail0
                        bl.append((h, cs, "tail", ric, BLK, 0))
                self.chunk_blocks[kk] = bl
        self.TOT = cur_blk
        self.IND_COLS = cur_ic
        self.TOTB = cur_bc  # == TOT

        # --- vectorized edge -> (slot, block) assignment ---
        order = np.lexsort((j, key))
        sk = key[order]
        newgrp = np.ones(len(sk), dtype=bool)
        newgrp[1:] = sk[1:] != sk[:-1]
        starts = np.flatnonzero(newgrp)
        lengths = np.diff(np.append(starts, len(sk)))
        rank_sorted = np.arange(len(sk)) - np.repeat(starts, lengths)
        rank = np.empty(len(sk), dtype=np.int64)
        rank[order] = rank_sorted  # rank within (core,k,half,j)

        capacity = Bfull[k, half, j] * BLK
        is_full = rank < capacity
        lo_pref = np.cumsum(leftover, axis=3) - leftover
        tail_rank = lo_pref[core, k, half, j] + (rank - capacity)

        # gather slot (s_global into idx16)
        sg_full = full_sg[k, half, j] + rank
        sg_tail = tail_sg[k, half] + tail_rank
        sg = np.where(is_full, sg_full, sg_tail)
        slot = np.where(is_full, rank % BLK, tail_rank % BLK)
        # ind-order block col (into wcol/nval) and window col
        bc_full = full_bc[k, half, j] + rank // BLK
        bc_tail_ = tail_bc[k, half] + tail_rank // BLK
        bc = np.where(is_full, bc_full, bc_tail_)
        wc = np.where(is_full, w32, w128)

        self.per_core = []
        for c in range(NC):
            m = core == c
            ncols = self.TOT * BLK // 16
            idx16 = np.zeros((16, ncols), dtype=np.int16)
            sgm = sg[m]
            vals = (rows[m] - half[m] * SPLIT).astype(np.int16)
            idx16[sgm % 16, sgm // 16] = vals
            wcol = np.zeros((BLK, self.TOTB), dtype=np.float32)
            nval = np.zeros((BLK, self.TOTB), dtype=np.float32)
            wcol[slot[m], bc[m]] = wc[m].astype(np.float32)
            nval[slot[m], bc[m]] = norm[m]
            self.per_core.append((idx16, wcol, nval))


def preprocess(cfg: Cfg, inputs):
    x = np.asarray(inputs["x"], dtype=np.float32)
    ei = np.asarray(inputs["edge_index"], dtype=np.int64)
    batch = np.asarray(inputs["batch"], dtype=np.int64)
    W1 = np.asarray(inputs["W1"], np.float32)
    b1 = np.asarray(inputs["b1"], np.float32)
    W2 = np.asarray(inputs["W2"], np.float32)
    b2 = np.asarray(inputs["b2"], np.float32)
    W3 = np.asarray(inputs["W3"], np.float32)
    b3 = np.asarray(inputs["b3"], np.float32)
    linW = np.asarray(inputs["linW"], np.float32)
    linb = np.asarray(inputs["linb"], np.float32)

    N, NC, NPC, PADN, CH, G = cfg.N, cfg.NC, cfg.NPC, cfg.PADN, cfg.CH, cfg.G
    src = np.concatenate([ei[0], np.arange(N, dtype=np.int64)])
    dst = np.concatenate([ei[1], np.arange(N, dtype=np.int64)])
    deg = np.bincount(dst, minlength=N).astype(np.float32)
    dinv = 1.0 / np.sqrt(deg)
    norm = (dinv[src] * dinv[dst]).astype(np.float32)

    L1 = LayerStruct(cfg, src, dst, norm, n_table_rows=N)
    r_of = (src // NPC) * PADN + (src % NPC)
    L2 = LayerStruct(cfg, r_of, dst, norm, n_table_rows=NC * PADN)

    cnt = np.maximum(np.bincount(batch, minlength=G), 1).astype(np.float32)
    coef = norm / cnt[batch[dst]]
    c_src = src // NPC
    kk = (src % NPC) >> 7
    ll = (src % NPC) & 127
    gg = batch[dst]
    flat = ((c_src * CH + kk) * 128 + ll) * G + gg
    C = np.bincount(flat, weights=coef.astype(np.float64), minlength=NC * CH * 128 * G)
    C = C.reshape(NC, CH * 128, G).astype(np.float32)

    w3 = (W3 @ linW).astype(np.float32)
    c_const = float(b3 @ linW[:, 0] + linb[0])
    empty = np.bincount(batch, minlength=G) == 0

    H = cfg.H
    bfnp = mybir.dt.np(mybir.dt.bfloat16)
    in_maps = []
    for c in range(NC):
        idx1, wcol1, nval1 = L1.per_core[c]
        idx2, wcol2, nval2 = L2.per_core[c]
        in_maps.append(
            {
                "xsh": x[c * NPC : (c + 1) * NPC, :].copy(),
                "W1": W1,
                "b1": b1.reshape(H, 1),
                "W2": W2,
                "b2": b2.reshape(2, H).T.copy(),
                "w3": w3.reshape(2, H).T.copy(),
                "idx1": idx1,
                "wcol1": wcol1,
                "nval1": nval1,
                "idx2": idx2,
                "wcol2": wcol2.astype(bfnp),
                "nval2": nval2.astype(bfnp),
                "C": C[c].astype(bfnp),
            }
        )
    host = dict(c_const=c_const, empty=empty, linb=float(linb[0]))
    return L1, L2, in_maps, host


def build_module(cfg: Cfg, L1: LayerStruct, L2: LayerStruct, single_core: bool = False, probe: str = ""):
    N, NC, NPC, PADN, CH, G = cfg.N, cfg.NC, cfg.NPC, cfg.PADN, cfg.CH, cfg.G
    FIN, H, H2 = cfg.FIN, cfg.H, cfg.H2
    f32 = mybir.dt.float32
    bf16 = mybir.dt.bfloat16
    i16 = mybir.dt.int16

    nc = bacc.Bacc("TRN2", debug=False, num_devices=1 if single_core else NC)
    xsh_t = nc.dram_tensor("xsh", [NPC, FIN], f32, kind="ExternalInput")
    W1_t = nc.dram_tensor("W1", [FIN, H], f32, kind="ExternalInput")
    b1_t = nc.dram_tensor("b1", [H, 1], f32, kind="ExternalInput")
    W2_t = nc.dram_tensor("W2", [H, H2], f32, kind="ExternalInput")
    b2_t = nc.dram_tensor("b2", [H, 2], f32, kind="ExternalInput")
    w3_t = nc.dram_tensor("w3", [H, 2], f32, kind="ExternalInput")
    idx1_t = nc.dram_tensor("idx1", [16, L1.TOT * 8], i16, kind="ExternalInput")
    wcol1_t = nc.dram_tensor("wcol1", [128, L1.TOTB], f32, kind="ExternalInput")
    nval1_t = nc.dram_tensor("nval1", [128, L1.TOTB], f32, kind="ExternalInput")
    idx2_t = nc.dram_tensor("idx2", [16, L2.TOT * 8], i16, kind="ExternalInput")
    wcol2_t = nc.dram_tensor("wcol2", [128, L2.TOTB], bf16, kind="ExternalInput")
    nval2_t = nc.dram_tensor("nval2", [128, L2.TOTB], bf16, kind="ExternalInput")
    C_t = nc.dram_tensor("C", [CH * 128, G], bf16, kind="ExternalInput")
    out_t = nc.dram_tensor("out", [G, 1], f32, kind="ExternalOutput")

    xloc = nc.dram_tensor("xloc", [NPC, FIN], f32)
    xfull = nc.dram_tensor("xfull", [N, FIN], f32, addr_space="Shared")
    h1sh = nc.dram_tensor("h1sh", [PADN, H], bf16)
    h1full = nc.dram_tensor("h1full", [NC * PADN, H], bf16, addr_space="Shared")

    with tile.TileContext(nc) as tc:
        with (
            tc.tile_pool(name="const", bufs=1) as cpool,
            tc.tile_pool(name="idxall", bufs=1) as idxallp,
            tc.tile_pool(name="slotd", bufs=1) as slotp,
            tc.tile_pool(name="ind", bufs=2) as indp,
            tc.tile_pool(name="msg", bufs=2) as msgp,
            tc.tile_pool(name="sb", bufs=2) as sbp,
            tc.tile_pool(name="qpool", bufs=1) as qpool,
            tc.tile_pool(name="zps", bufs=2, space="PSUM") as zpsp,
            tc.tile_pool(name="hps", bufs=2, space="PSUM") as hpsp,
            tc.tile_pool(name="tps", bufs=1, space="PSUM") as tpsp,
            tc.tile_pool(name="qps", bufs=1, space="PSUM") as qpsp,
            tc.tile_pool(name="pps", bufs=1, space="PSUM") as ppsp,
            tc.tile_pool(name="scr", bufs=1, space="PSUM") as scrp,
        ):
            zero_sb = cpool.tile([128, 128], f32)
            nc.vector.memset(zero_sb[:], 0.0)
            zero_bf = cpool.tile([128, 1], bf16)
            nc.vector.memset(zero_bf[:], 0.0)
            ident = cpool.tile([128, 128], f32)
            make_identity(nc, ident[:])
            W1_sb = cpool.tile([FIN, H], f32)
            nc.sync.dma_start(out=W1_sb[:], in_=W1_t[:, :])
            b1_sb = cpool.tile([H, 1], f32)
            nc.sync.dma_start(out=b1_sb[:], in_=b1_t[:, :])
            W2_sb = cpool.tile([H, H2], f32)
            nc.sync.dma_start(out=W2_sb[:], in_=W2_t[:, :])
            b2_sb = cpool.tile([H, 2], f32)
            nc.sync.dma_start(out=b2_sb[:], in_=b2_t[:, :])
            w3_sb = cpool.tile([H, 2], f32)
            nc.sync.dma_start(out=w3_sb[:], in_=w3_t[:, :])
            scr_ps = scrp.tile([1, 1], f32, space="PSUM")
            q_sb = qpool.tile([128, CH], bf16)
            pool_ps = ppsp.tile([G, 1], f32, space="PSUM")

            # iota ramps 0..31 and 0..127 (f32, same value in every partition)
            ramps = {}
            for dt_, dname in ((f32, "f"), (bf16, "b")):
                r32 = cpool.tile([128, W], dt_, name=f"ramp32{dname}")
                nc.gpsimd.iota(
                    r32[:], [[1, W]], channel_multiplier=0,
                    allow_small_or_imprecise_dtypes=True,
                )
                r128 = cpool.tile([128, BLK], dt_, name=f"ramp128{dname}")
                nc.gpsimd.iota(
                    r128[:], [[1, BLK]], channel_multiplier=0,
                    allow_small_or_imprecise_dtypes=True,
                )
                ramps[dt_] = (r32, r128)

            def absorb(dep_ap):
                kdim = dep_ap.shape[0]
                zt = zero_bf if dep_ap.dtype == bf16 else zero_sb
                nc.tensor.matmul(
                    scr_ps[:], lhsT=zt[:kdim, :1], rhs=dep_ap, start=True, stop=True
                )

            absorb(zero_sb[:, :1])
            for cst in (ident, W1_sb, b1_sb, W2_sb, b2_sb, w3_sb):
                absorb(cst[:, :1])
            act_scr = cpool.tile([H, 3], f32)
            nc.scalar.copy(act_scr[:, 0:1], b1_sb[:, :1])
            nc.scalar.copy(act_scr[:, 1:2], b2_sb[:, 0:1])
            nc.scalar.copy(act_scr[:, 2:3], b2_sb[:, 1:2])

            # ---- x AllGather (replicate shards) ----
            # collectives cannot read IO tensors: stage the shard into an
            # internal DRAM tensor first
            nc.sync.dma_start(out=xloc[:, :], in_=xsh_t[:, :])
            if single_core:
                nc.sync.dma_start(out=xfull[0:NPC, :], in_=xloc[:, :])
            else:
                nc.gpsimd.collective_compute(
                    "AllGather",
                    mybir.AluOpType.bypass,
                    replica_groups=[list(range(NC))],
                    ins=[xloc[:, :]],
                    outs=[xfull[:, :]],
                )

            # ---- per-layer slot data ----
            def load_layer_inputs(LS, idx_t, wcol_t, nval_t, tag, dt_):
                idx_sb = idxallp.tile([128, LS.TOT * 8], i16, tag=f"idx{tag}")
                for r in range(8):
                    nc.sync.dma_start(
                        out=idx_sb[16 * r : 16 * (r + 1), :], in_=idx_t[:, :]
                    )
                wcol_sb = slotp.tile([128, LS.TOTB], dt_, tag=f"wc{tag}")
                nc.sync.dma_start(out=wcol_sb[:], in_=wcol_t[:, :])
                nval_sb = slotp.tile([128, LS.TOTB], dt_, tag=f"nv{tag}")
                nc.sync.dma_start(out=nval_sb[:], in_=nval_t[:, :])
                return idx_sb, wcol_sb, nval_sb

            def sparse_layer(LS: LayerStruct, F, layer_sbs, lo_ap, hi_ap, consume_chunk, dt_):
                idx_sb, wcol_sb, nval_sb = layer_sbs
                ramp32, ramp128 = ramps[dt_]
                for g_i, g in enumerate(LS.groups):
                    fb = g["first_blk"]
                    nlo, nhi = g["lo_cnt"], g["hi_cnt"]
                    msg_tiles = {}
                    for h, cnt_, table_ap in ((0, nlo, lo_ap), (1, nhi, hi_ap)):
                        if cnt_ == 0:
                            continue
                        nidx = cnt_ * BLK
                        col0 = (fb + (nlo if h else 0)) * 8
                        msg = msgp.tile([128, cnt_ * F], dt_, tag=f"msg_{h}")
                        if "nogather" in probe:
                            nc.vector.memset(msg[:, :1], 0.0)
                        else:
                            nc.gpsimd.dma_gather(
                                msg[:].rearrange("p (b f) -> p b f", b=cnt_),
                                table_ap,
                                idx_sb[:, col0 : col0 + nidx // 16],
                                num_idxs=nidx,
                                num_idxs_reg=nidx,
                                elem_size=F,
                                single_packet=False,
                            )
                        msg_tiles[h] = msg
                    # ---- build ind on DVE: fulls then tails ----
                    nf, nt = g["nf"], g["nt"]
                    fbc, tbc = g["first_bc"], g["tail_bc0"]
                    ind_f = indp.tile([128, max(nf, 1) * W], dt_, tag="indf")
                    if nf and "noind" not in probe:
                        pred = (
                            ramp32[:]
                            .rearrange("p (o w) -> p o w", o=1)
                            .broadcast_to([128, nf, W])
                        )
                        wc = (
                            wcol_sb[:, fbc : fbc + nf]
                            .rearrange("p (b o) -> p b o", o=1)
                            .broadcast_to([128, nf, W])
                        )
                        nv = (
                            nval_sb[:, fbc : fbc + nf]
                            .rearrange("p (b o) -> p b o", o=1)
                            .broadcast_to([128, nf, W])
                        )
                        nc.vector.tensor_tensor(
                            out=ind_f[:].rearrange("p (b w) -> p b w", b=nf),
                            in0=pred, in1=wc, op=mybir.AluOpType.is_equal,
                        )
                        nc.vector.tensor_tensor(
                            out=ind_f[:].rearrange("p (b w) -> p b w", b=nf),
                            in0=ind_f[:].rearrange("p (b w) -> p b w", b=nf),
                            in1=nv, op=mybir.AluOpType.mult,
                        )
                    elif nf:
                        nc.vector.memset(ind_f[:, :1], 0.0)
                    ind_t = indp.tile([128, max(nt, 1) * BLK], dt_, tag="indt")
                    if nt and "noind" not in probe:
                        pred = (
                            ramp128[:]
                            .rearrange("p (o w) -> p o w", o=1)
                            .broadcast_to([128, nt, BLK])
                        )
                        wc = (
                            wcol_sb[:, tbc : tbc + nt]
                            .rearrange("p (b o) -> p b o", o=1)
                            .broadcast_to([128, nt, BLK])
                        )
                        nv = (
                            nval_sb[:, tbc : tbc + nt]
                            .rearrange("p (b o) -> p b o", o=1)
                            .broadcast_to([128, nt, BLK])
                        )
                        nc.vector.tensor_tensor(
                            out=ind_t[:].rearrange("p (b w) -> p b w", b=nt),
                            in0=pred, in1=wc, op=mybir.AluOpType.is_equal,
                        )
                        nc.vector.tensor_tensor(
                            out=ind_t[:].rearrange("p (b w) -> p b w", b=nt),
                            in0=ind_t[:].rearrange("p (b w) -> p b w", b=nt),
                            in1=nv, op=mybir.AluOpType.mult,
                        )
                    elif nt:
                        nc.vector.memset(ind_t[:, :1], 0.0)
                    for dep in (*msg_tiles.values(), ind_f, ind_t):
                        if "noabsorb" in probe:
                            break
                        absorb(dep[:, :1])
                    for kk in g["chunks"]:
                        blocks = LS.chunk_blocks[kk]
                        zps = zpsp.tile([128, 128], f32, space="PSUM", tag="z")
                        nc.tensor.matmul(
                            zps[:F, :], lhsT=zero_sb[:, :F], rhs=zero_sb[:, :],
                            start=True, stop=False,
                        )
                        for bi, (h, cs, kind, ric, width, ooff) in enumerate(blocks):
                            if "noblocks" in probe:
                                break
                            last = bi == len(blocks) - 1
                            msg = msg_tiles[h]
                            rhs_tile = ind_f if kind == "full" else ind_t
                            nc.tensor.matmul(
                                zps[:F, ooff : ooff + width],
                                lhsT=msg[:, cs * F : (cs + 1) * F],
                                rhs=rhs_tile[:, ric : ric + width],
                                start=False,
                                stop=last,
                            )
                        z_sb = sbp.tile([F, 128], f32, tag="z_sb")
                        nc.vector.tensor_copy(out=z_sb[:], in_=zps[:F, :])
                        consume_chunk(kk, z_sb)

            # ---- Layer 1 ----
            def l1_chunk(kk, z_sb):
                absorb(z_sb[:, :1])
                hps = hpsp.tile([H, 128], f32, space="PSUM", tag="h")
                nc.tensor.matmul(hps[:], lhsT=W1_sb[:], rhs=z_sb[:FIN, :], start=True, stop=True)
                h1T = sbp.tile([H, 128], f32, tag="h1T")
                nc.scalar.activation(
                    h1T[:], hps[:], mybir.ActivationFunctionType.Relu, bias=b1_sb[:, :]
                )
                absorb(h1T[:, :1])
                tps = tpsp.tile([128, H], f32, space="PSUM", tag="t")
                nc.tensor.transpose(out=tps[:], in_=h1T[:], identity=ident[:])
                h1n = sbp.tile([128, H], bf16, tag="h1n")
                nc.vector.tensor_copy(out=h1n[:], in_=tps[:])
                nc.sync.dma_start(out=h1sh[kk * 128 : (kk + 1) * 128, :], in_=h1n[:])

            l1_sbs = load_layer_inputs(L1, idx1_t, wcol1_t, nval1_t, "1", f32)
            sparse_layer(
                L1, FIN, l1_sbs,
                xfull[0 : L1.n_lo_rows, :],
                xfull[L1.n_lo_rows : N, :] if L1.n_hi_rows else xfull[0:1, :],
                l1_chunk,
                f32,
            )

            # ---- AllGather h1 ----
            if single_core:
                nc.sync.dma_start(out=h1full[0:PADN, :], in_=h1sh[:, :])
            else:
                nc.gpsimd.collective_compute(
                    "AllGather",
                    mybir.AluOpType.bypass,
                    replica_groups=[list(range(NC))],
                    ins=[h1sh[:, :]],
                    outs=[h1full[:, :]],
                )

            # ---- Layer 2 + head ----
            def l2_chunk(kk, z_sb):
                absorb(z_sb[:, :1])
                h2T_halves = []
                for half_i in range(2):
                    hps = hpsp.tile([H, 128], f32, space="PSUM", tag="h")
                    nc.tensor.matmul(
                        hps[:],
                        lhsT=W2_sb[:, half_i * H : (half_i + 1) * H],
                        rhs=z_sb[:],
                        start=True,
                        stop=True,
                    )
                    h2T = sbp.tile([H, 128], f32, tag=f"h2T{half_i}")
                    nc.scalar.activation(
                        h2T[:],
                        hps[:],
                        mybir.ActivationFunctionType.Relu,
                        bias=b2_sb[:, half_i : half_i + 1],
                    )
                    h2T_halves.append(h2T)
                absorb(h2T_halves[0][:, :1])
                absorb(h2T_halves[1][:, :1])
                qps = qpsp.tile([128, 1], f32, space="PSUM", tag="q")
                for half_i in range(2):
                    nc.tensor.matmul(
                        qps[:],
                        lhsT=h2T_halves[half_i][:],
                        rhs=w3_sb[:, half_i : half_i + 1],
                        start=half_i == 0,
                        stop=half_i == 1,
                    )
                nc.vector.tensor_copy(out=q_sb[:, kk : kk + 1], in_=qps[:])
                Cs = sbp.tile([128, G], bf16, tag="Cs")
                nc.sync.dma_start(out=Cs[:], in_=C_t[kk * 128 : (kk + 1) * 128, :])
                absorb(Cs[:, :1])
                nc.tensor.matmul(
                    pool_ps[:],
                    lhsT=Cs[:],
                    rhs=q_sb[:, kk : kk + 1],
                    start=kk == 0,
                    stop=kk == CH - 1,
                )

            l2_sbs = load_layer_inputs(L2, idx2_t, wcol2_t, nval2_t, "2", bf16)
            sparse_layer(
                L2, H, l2_sbs,
                h1full[0 : L2.n_lo_rows, :],
                h1full[L2.n_lo_rows : NC * PADN, :] if L2.n_hi_rows else h1full[0:1, :],
                l2_chunk,
                bf16,
            )

            pool_sb = sbp.tile([G, 1], f32, tag="pool")
            nc.vector.tensor_copy(out=pool_sb[:], in_=pool_ps[:])
            nc.sync.dma_start(out=out_t[:, :], in_=pool_sb[:])

    nc.compile()
    return nc


def postprocess(cfg: Cfg, results, host):
    out = np.zeros((cfg.G, 1), dtype=np.float64)
    for r in results:
        out += r["out"].astype(np.float64)
    out += host["c_const"]
    out[host["empty"], 0] = host["linb"]
    return out.astype(np.float32)


from concourse import bass_utils as _bass_utils


def kernel(**inputs) -> np.ndarray:
    cfg = Cfg()
    L1, L2, in_maps, host = preprocess(cfg, inputs)
    nc = build_module(cfg, L1, L2)
    res = _bass_utils.run_bass_kernel_spmd(nc, in_maps, core_ids=list(range(cfg.NC)))
    return postprocess(cfg, res.results, host)


# revision 3
# speedup vs baseline: 1.3352x; 1.2175x over previous
"""GCN (3-layer + mean-pool head) on 8 Trainium2 cores — v3 = v2 + bf16 L2 path.

bf16: h1 table (halves L2 gather traffic + h1 AllGather), L2 msg/ind matmuls
(4x PE stream rate vs fp32), C matrix + q (halves head DMA). L1 stays fp32.
"""

_V2_DOC = """GCN (3-layer + mean-pool head) on 8 Trainium2 cores — v2, slim inputs.

Differences from v1:
  - ind matrices built ON DEVICE from per-slot (wcol, norm) arrays via
    iota-ramp is_equal + multiply (upload 1.2MB/layer instead of 30MB).
  - idx uploaded un-tiled [16, TOT*8] and replicated to 128 partitions on
    device (0.3MB instead of 2.4MB per layer).
  - x uploaded sharded [NPC, FIN] per core and AllGathered on device
    (1.6MB instead of 12.8MB per core).
Per-group block enumeration: gather order per half = fulls then tails;
ind columns: fulls region (width 32 each) then tails region (width 128).
"""  # noqa: E501

from dataclasses import dataclass
import numpy as np

import concourse.bass as bass
import concourse.bacc as bacc
import concourse.mybir as mybir
import concourse.tile as tile
from concourse.masks import make_identity

BLK = 128
W = 32
NW = 4


@dataclass
class Cfg:
    N: int = 50000
    E: int = 1000000
    G: int = 128
    FIN: int = 64
    H: int = 128
    H2: int = 256
    NC: int = 8
    CG: int = 4
    SPLIT: int = 32768

    @property
    def NPC(self):
        assert self.N % self.NC == 0
        return self.N // self.NC

    @property
    def CH(self):
        return (self.NPC + 127) // 128

    @property
    def PADN(self):
        return self.CH * 128

    @property
    def NG(self):
        return (self.CH + self.CG - 1) // self.CG


def _ceil_div(a, b):
    return -(-a // b)


class LayerStruct:
    """Block structure shared across cores + per-core compact arrays.

    Per group g (CG chunks):
      gather order: half h: [fulls(k asc, j asc, b), tails(k asc, b)] -> cs
      ind columns:  fulls region [fulls h0 ++ fulls h1] (width 32 each),
                    tails region [tails h0 ++ tails h1] (width 128 each)
      idx16 columns: group base gcol0 = first_blk*8; h0 blocks then h1 blocks
        in gather order, 8 int16 cols per block.
    Per-core arrays:
      idx16 [16, TOT*8]   wrapped gather indices (block-major in gather order)
      wcol  [128, TOT_ind] f32 window col per slot (ind order: per group fulls
                           then tails, concatenated over groups)
      nval  [128, TOT_ind] f32 norm per slot (0 padding)
    """

    def __init__(self, cfg: Cfg, rows, dst, norm, n_table_rows):
        NC, CH, NPC, CG, SPLIT = cfg.NC, cfg.CH, cfg.NPC, cfg.CG, cfg.SPLIT
        core = dst // NPC
        l = dst - core * NPC
        k = l >> 7
        j = (l >> 5) & 3
        w32 = l & 31
        w128 = l & 127
        half = (rows >= SPLIT).astype(np.int64)
        self.n_lo_rows = min(SPLIT, n_table_rows)
        self.n_hi_rows = max(0, n_table_rows - SPLIT)

        key = (((core * CH + k) * 2 + half) * NW + j)
        counts = np.bincount(key, minlength=NC * CH * 2 * NW).reshape(NC, CH, 2, NW)
        Bfull = (counts // BLK).max(axis=0)  # [CH, 2, NW]
        leftover = counts - np.minimum(counts, Bfull[None] * BLK)
        tail_cnt = leftover.sum(axis=3)  # [NC, CH, 2]
        Btail = _ceil_div(tail_cnt, BLK).max(axis=0)  # [CH, 2]
        self.Bfull, self.Btail = Bfull, Btail

        # --- enumerate blocks ---
        # per (k,h,j): gather cs base; per (k,h): tail cs base
        # per block: ind column offset (fulls then tails region per group)
        full_cs = np.zeros((CH, 2, NW), dtype=np.int64)  # cs of first full blk
        tail_cs = np.zeros((CH, 2), dtype=np.int64)
        full_sg = np.zeros((CH, 2, NW), dtype=np.int64)  # global gather slot base
        tail_sg = np.zeros((CH, 2), dtype=np.int64)
        full_ic = np.zeros((CH, 2, NW), dtype=np.int64)  # ind col offset (global)
        tail_ic = np.zeros((CH, 2), dtype=np.int64)
        # ind-order column index (into wcol/nval [*, TOT_ind]) per block
        full_bc = np.zeros((CH, 2, NW), dtype=np.int64)
        tail_bc = np.zeros((CH, 2), dtype=np.int64)

        self.groups = []
        self.chunk_blocks = [None] * CH  # list of (h, cs, ric_kind, roff, width, ooff)
        cur_blk = 0  # global block counter (gather order, h-grouped per group)
        cur_ic = 0  # global ind col counter
        cur_bc = 0  # global ind-order block col counter
        for g in range(cfg.NG):
            ks = list(range(g * CG, min((g + 1) * CG, CH)))
            first_blk = cur_blk
            # gather order per half
            half_cnt = [0, 0]
            for h in (0, 1):
                cs = 0
                for kk in ks:
                    for jj in range(NW):
                        full_cs[kk, h, jj] = cs
                        cs += Bfull[kk, h, jj]
                for kk in ks:
                    tail_cs[kk, h] = cs
                    cs += Btail[kk, h]
                half_cnt[h] = cs
            nlo, nhi = half_cnt
            for h in (0, 1):
                base = first_blk + (nlo if h else 0)
                for kk in ks:
                    for jj in range(NW):
                        full_sg[kk, h, jj] = (base + full_cs[kk, h, jj]) * BLK
                    tail_sg[kk, h] = (base + tail_cs[kk, h]) * BLK
            # ind columns: fulls h0 ++ fulls h1, then tails h0 ++ tails h1
            first_ic = cur_ic
            first_bc = cur_bc
            nf = 0
            for h in (0, 1):
                for kk in ks:
                    for jj in range(NW):
                        full_ic[kk, h, jj] = cur_ic
                        full_bc[kk, h, jj] = cur_bc
                        cur_ic += Bfull[kk, h, jj] * W
                        cur_bc += Bfull[kk, h, jj]
                        nf += Bfull[kk, h, jj]
            ic_tail0 = cur_ic
            bc_tail0 = cur_bc
            nt = 0
            for h in (0, 1):
                for kk in ks:
                    tail_ic[kk, h] = cur_ic
                    tail_bc[kk, h] = cur_bc
                    cur_ic += Btail[kk, h] * BLK
                    cur_bc += Btail[kk, h]
                    nt += Btail[kk, h]
            cur_blk += nlo + nhi
            self.groups.append(
                dict(
                    chunks=ks,
                    first_blk=first_blk,
                    lo_cnt=nlo,
                    hi_cnt=nhi,
                    nf=nf,
                    nt=nt,
                    first_ic=first_ic,  # fulls ind region start (global col)
                    tail_ic0=ic_tail0,  # tails ind region start
                    first_bc=first_bc,  # fulls block-col start in wcol/nval
                    tail_bc0=bc_tail0,
                )
            )
            # per-chunk emission metadata
            for kk in ks:
                bl = []
                for h in (0, 1):
                    for jj in range(NW):
                        for b in range(Bfull[kk, h, jj]):
                            cs = full_cs[kk, h, jj] + b
                            ric = full_ic[kk, h, jj] + b * W - first_ic
                            bl.append((h, cs, "full", ric, W, jj * W))
                    for b in range(Btail[kk, h]):
                        cs = tail_cs[kk, h] + b
                        ric = tail_ic[kk, h] + b * BLK - ic_tail0
                        bl.append((h, cs, "tail", ric, BLK, 0))
                self.chunk_blocks[kk] = bl
        self.TOT = cur_blk
        self.IND_COLS = cur_ic
        self.TOTB = cur_bc  # == TOT

        # --- vectorized edge -> (slot, block) assignment ---
        order = np.lexsort((j, key))
        sk = key[order]
        newgrp = np.ones(len(sk), dtype=bool)
        newgrp[1:] = sk[1:] != sk[:-1]
        starts = np.flatnonzero(newgrp)
        lengths = np.diff(np.append(starts, len(sk)))
        rank_sorted = np.arange(len(sk)) - np.repeat(starts, lengths)
        rank = np.empty(len(sk), dtype=np.int64)
        rank[order] = rank_sorted  # rank within (core,k,half,j)

        capacity = Bfull[k, half, j] * BLK
        is_full = rank < capacity
        lo_pref = np.cumsum(leftover, axis=3) - leftover
        tail_rank = lo_pref[core, k, half, j] + (rank - capacity)

        # gather slot (s_global into idx16)
        sg_full = full_sg[k, half, j] + rank
        sg_tail = tail_sg[k, half] + tail_rank
        sg = np.where(is_full, sg_full, sg_tail)
        slot = np.where(is_full, rank % BLK, tail_rank % BLK)
        # ind-order block col (into wcol/nval) and window col
        bc_full = full_bc[k, half, j] + rank // BLK
        bc_tail_ = tail_bc[k, half] + tail_rank // BLK
        bc = np.where(is_full, bc_full, bc_tail_)
        wc = np.where(is_full, w32, w128)

        self.per_core = []
        for c in range(NC):
            m = core == c
            ncols = self.TOT * BLK // 16
            idx16 = np.zeros((16, ncols), dtype=np.int16)
            sgm = sg[m]
            vals = (rows[m] - half[m] * SPLIT).astype(np.int16)
            idx16[sgm % 16, sgm // 16] = vals
            wcol = np.zeros((BLK, self.TOTB), dtype=np.float32)
            nval = np.zeros((BLK, self.TOTB), dtype=np.float32)
            wcol[slot[m], bc[m]] = wc[m].astype(np.float32)
            nval[slot[m], bc[m]] = norm[m]
            self.per_core.append((idx16, wcol, nval))


def preprocess(cfg: Cfg, inputs):
    x = np.asarray(inputs["x"], dtype=np.float32)
    ei = np.asarray(inputs["edge_index"], dtype=np.int64)
    batch = np.asarray(inputs["batch"], dtype=np.int64)
    W1 = np.asarray(inputs["W1"], np.float32)
    b1 = np.asarray(inputs["b1"], np.float32)
    W2 = np.asarray(inputs["W2"], np.float32)
    b2 = np.asarray(inputs["b2"], np.float32)
    W3 = np.asarray(inputs["W3"], np.float32)
    b3 = np.asarray(inputs["b3"], np.float32)
    linW = np.asarray(inputs["linW"], np.float32)
    linb = np.asarray(inputs["linb"], np.float32)

    N, NC, NPC, PADN, CH, G = cfg.N, cfg.NC, cfg.NPC, cfg.PADN, cfg.CH, cfg.G
    src = np.concatenate([ei[0], np.arange(N, dtype=np.int64)])
    dst = np.concatenate([ei[1], np.arange(N, dtype=np.int64)])
    deg = np.bincount(dst, minlength=N).astype(np.float32)
    dinv = 1.0 / np.sqrt(deg)
    norm = (dinv[src] * dinv[dst]).astype(np.float32)

    L1 = LayerStruct(cfg, src, dst, norm, n_table_rows=N)
    r_of = (src // NPC) * PADN + (src % NPC)
    L2 = LayerStruct(cfg, r_of, dst, norm, n_table_rows=NC * PADN)

    cnt = np.maximum(np.bincount(batch, minlength=G), 1).astype(np.float32)
    coef = norm / cnt[batch[dst]]
    c_src = src // NPC
    kk = (src % NPC) >> 7
    ll = (src % NPC) & 127
    gg = batch[dst]
    flat = ((c_src * CH + kk) * 128 + ll) * G + gg
    C = np.bincount(flat, weights=coef.astype(np.float64), minlength=NC * CH * 128 * G)
    C = C.reshape(NC, CH * 128, G).astype(np.float32)

    w3 = (W3 @ linW).astype(np.float32)
    c_const = float(b3 @ linW[:, 0] + linb[0])
    empty = np.bincount(batch, minlength=G) == 0

    H = cfg.H
    bfnp = mybir.dt.np(mybir.dt.bfloat16)
    in_maps = []
    for c in range(NC):
        idx1, wcol1, nval1 = L1.per_core[c]
        idx2, wcol2, nval2 = L2.per_core[c]
        in_maps.append(
            {
                "xsh": x[c * NPC : (c + 1) * NPC, :].copy(),
                "W1": W1,
                "b1": b1.reshape(H, 1),
                "W2": W2,
                "b2": b2.reshape(2, H).T.copy(),
                "w3": w3.reshape(2, H).T.copy(),
                "idx1": idx1,
                "wcol1": wcol1,
                "nval1": nval1,
                "idx2": idx2,
                "wcol2": wcol2.astype(bfnp),
                "nval2": nval2.astype(bfnp),
                "C": C[c].astype(bfnp),
            }
        )
    host = dict(c_const=c_const, empty=empty, linb=float(linb[0]))
    return L1, L2, in_maps, host


def build_module(cfg: Cfg, L1: LayerStruct, L2: LayerStruct, single_core: bool = False, probe: str = ""):
    N, NC, NPC, PADN, CH, G = cfg.N, cfg.NC, cfg.NPC, cfg.PADN, cfg.CH, cfg.G
    FIN, H, H2 = cfg.FIN, cfg.H, cfg.H2
    f32 = mybir.dt.float32
    bf16 = mybir.dt.bfloat16
    i16 = mybir.dt.int16

    nc = bacc.Bacc("TRN2", debug=False, num_devices=1 if single_core else NC)
    xsh_t = nc.dram_tensor("xsh", [NPC, FIN], f32, kind="ExternalInput")
    W1_t = nc.dram_tensor("W1", [FIN, H], f32, kind="ExternalInput")
    b1_t = nc.dram_tensor("b1", [H, 1], f32, kind="ExternalInput")
    W2_t = nc.dram_tensor("W2", [H, H2], f32, kind="ExternalInput")
    b2_t = nc.dram_tensor("b2", [H, 2], f32, kind="ExternalInput")
    w3_t = nc.dram_tensor("w3", [H, 2], f32, kind="ExternalInput")
    idx1_t = nc.dram_tensor("idx1", [16, L1.TOT * 8], i16, kind="ExternalInput")
    wcol1_t = nc.dram_tensor("wcol1", [128, L1.TOTB], f32, kind="ExternalInput")
    nval1_t = nc.dram_tensor("nval1", [128, L1.TOTB], f32, kind="ExternalInput")
    idx2_t = nc.dram_tensor("idx2", [16, L2.TOT * 8], i16, kind="ExternalInput")
    wcol2_t = nc.dram_tensor("wcol2", [128, L2.TOTB], bf16, kind="ExternalInput")
    nval2_t = nc.dram_tensor("nval2", [128, L2.TOTB], bf16, kind="ExternalInput")
    C_t = nc.dram_tensor("C", [CH * 128, G], bf16, kind="ExternalInput")
    out_t = nc.dram_tensor("out", [G, 1], f32, kind="ExternalOutput")

    xloc = nc.dram_tensor("xloc", [NPC, FIN], f32)
    xfull = nc.dram_tensor("xfull", [N, FIN], f32, addr_space="Shared")
    h1sh = nc.dram_tensor("h1sh", [PADN, H], bf16)
    h1full = nc.dram_tensor("h1full", [NC * PADN, H], bf16, addr_space="Shared")

    with tile.TileContext(nc) as tc:
        with (
            tc.tile_pool(name="const", bufs=1) as cpool,
            tc.tile_pool(name="idxall", bufs=1) as idxallp,
            tc.tile_pool(name="slotd", bufs=1) as slotp,
            tc.tile_pool(name="ind", bufs=2) as indp,
            tc.tile_pool(name="msg", bufs=2) as msgp,
            tc.tile_pool(name="sb", bufs=2) as sbp,
            tc.tile_pool(name="qpool", bufs=1) as qpool,
            tc.tile_pool(name="zps", bufs=2, space="PSUM") as zpsp,
            tc.tile_pool(name="hps", bufs=2, space="PSUM") as hpsp,
            tc.tile_pool(name="tps", bufs=1, space="PSUM") as tpsp,
            tc.tile_pool(name="qps", bufs=1, space="PSUM") as qpsp,
            tc.tile_pool(name="pps", bufs=1, space="PSUM") as ppsp,
            tc.tile_pool(name="scr", bufs=1, space="PSUM") as scrp,
        ):
            zero_sb = cpool.tile([128, 128], f32)
            nc.vector.memset(zero_sb[:], 0.0)
            zero_bf = cpool.tile([128, 1], bf16)
            nc.vector.memset(zero_bf[:], 0.0)
            ident = cpool.tile([128, 128], f32)
            make_identity(nc, ident[:])
            W1_sb = cpool.tile([FIN, H], f32)
            nc.sync.dma_start(out=W1_sb[:], in_=W1_t[:, :])
            b1_sb = cpool.tile([H, 1], f32)
            nc.sync.dma_start(out=b1_sb[:], in_=b1_t[:, :])
            W2_sb = cpool.tile([H, H2], f32)
            nc.sync.dma_start(out=W2_sb[:], in_=W2_t[:, :])
            b2_sb = cpool.tile([H, 2], f32)
            nc.sync.dma_start(out=b2_sb[:], in_=b2_t[:, :])
            w3_sb = cpool.tile([H, 2], f32)
            nc.sync.dma_start(out=w3_sb[:], in_=w3_t[:, :])
            scr_ps = scrp.tile([1, 1], f32, space="PSUM")
            q_sb = qpool.tile([128, CH], bf16)
            pool_ps = ppsp.tile([G, 1], f32, space="PSUM")
            # whole C matrix resident: one DMA instead of 49 small ones
            Call = qpool.tile([128, CH * G], bf16, name="Call")
            nc.sync.dma_start(
                out=Call[:].rearrange("p (k g) -> p k g", k=CH),
                in_=C_t[:, :].rearrange("(k p) g -> p k g", k=CH),
            )

            # iota ramps 0..31 and 0..127 (f32, same value in every partition)
            ramps = {}
            for dt_, dname in ((f32, "f"), (bf16, "b")):
                r32 = cpool.tile([128, W], dt_, name=f"ramp32{dname}")
                nc.gpsimd.iota(
                    r32[:], [[1, W]], channel_multiplier=0,
                    allow_small_or_imprecise_dtypes=True,
                )
                r128 = cpool.tile([128, BLK], dt_, name=f"ramp128{dname}")
                nc.gpsimd.iota(
                    r128[:], [[1, BLK]], channel_multiplier=0,
                    allow_small_or_imprecise_dtypes=True,
                )
                ramps[dt_] = (r32, r128)

            def absorb(dep_ap):
                kdim = dep_ap.shape[0]
                zt = zero_bf if dep_ap.dtype == bf16 else zero_sb
                nc.tensor.matmul(
                    scr_ps[:], lhsT=zt[:kdim, :1], rhs=dep_ap, start=True, stop=True
                )

            absorb(zero_sb[:, :1])
            for cst in (ident, W1_sb, b1_sb, W2_sb, b2_sb, w3_sb):
                absorb(cst[:, :1])
            act_scr = cpool.tile([H, 3], f32)
            nc.scalar.copy(act_scr[:, 0:1], b1_sb[:, :1])
            nc.scalar.copy(act_scr[:, 1:2], b2_sb[:, 0:1])
            nc.scalar.copy(act_scr[:, 2:3], b2_sb[:, 1:2])
            absorb(Call[:, :1])

            # ---- x AllGather (replicate shards) ----
            # collectives cannot read IO tensors: stage the shard into an
            # internal DRAM tensor first
            nc.sync.dma_start(out=xloc[:, :], in_=xsh_t[:, :])
            if single_core:
                nc.sync.dma_start(out=xfull[0:NPC, :], in_=xloc[:, :])
            else:
                nc.gpsimd.collective_compute(
                    "AllGather",
                    mybir.AluOpType.bypass,
                    replica_groups=[list(range(NC))],
                    ins=[xloc[:, :]],
                    outs=[xfull[:, :]],
                )

            # ---- per-layer slot data ----
            def load_layer_inputs(LS, idx_t, wcol_t, nval_t, tag, dt_):
                idx_sb = idxallp.tile([128, LS.TOT * 8], i16, tag=f"idx{tag}")
                for r in range(8):
                    nc.sync.dma_start(
                        out=idx_sb[16 * r : 16 * (r + 1), :], in_=idx_t[:, :]
                    )
                wcol_sb = slotp.tile([128, LS.TOTB], dt_, tag=f"wc{tag}")
                nc.sync.dma_start(out=wcol_sb[:], in_=wcol_t[:, :])
                nval_sb = slotp.tile([128, LS.TOTB], dt_, tag=f"nv{tag}")
                nc.sync.dma_start(out=nval_sb[:], in_=nval_t[:, :])
                return idx_sb, wcol_sb, nval_sb

            def sparse_layer(LS: LayerStruct, F, layer_sbs, lo_ap, hi_ap, consume_chunk, dt_):
                idx_sb, wcol_sb, nval_sb = layer_sbs
                ramp32, ramp128 = ramps[dt_]
                for g_i, g in enumerate(LS.groups):
                    fb = g["first_blk"]
                    nlo, nhi = g["lo_cnt"], g["hi_cnt"]
                    msg_tiles = {}
                    for h, cnt_, table_ap in ((0, nlo, lo_ap), (1, nhi, hi_ap)):
                        if cnt_ == 0:
                            continue
                        nidx = cnt_ * BLK
                        col0 = (fb + (nlo if h else 0)) * 8
                        msg = msgp.tile([128, cnt_ * F], dt_, tag=f"msg_{h}")
                        if "nogather" in probe:
                            nc.vector.memset(msg[:, :1], 0.0)
                        else:
                            nc.gpsimd.dma_gather(
                                msg[:].rearrange("p (b f) -> p b f", b=cnt_),
                                table_ap,
                                idx_sb[:, col0 : col0 + nidx // 16],
                                num_idxs=nidx,
                                num_idxs_reg=nidx,
                                elem_size=F,
                                single_packet=False,
                            )
                        msg_tiles[h] = msg
                    # ---- build ind on DVE: fulls then tails ----
                    nf, nt = g["nf"], g["nt"]
                    fbc, tbc = g["first_bc"], g["tail_bc0"]
                    ind_f = indp.tile([128, max(nf, 1) * W], dt_, tag="indf")
                    if nf and "noind" not in probe:
                        pred = (
                            ramp32[:]
                            .rearrange("p (o w) -> p o w", o=1)
                            .broadcast_to([128, nf, W])
                        )
                        wc = (
                            wcol_sb[:, fbc : fbc + nf]
                            .rearrange("p (b o) -> p b o", o=1)
                            .broadcast_to([128, nf, W])
                        )
                        nv = (
                            nval_sb[:, fbc : fbc + nf]
                            .rearrange("p (b o) -> p b o", o=1)
                            .broadcast_to([128, nf, W])
                        )
                        nc.vector.tensor_tensor(
                            out=ind_f[:].rearrange("p (b w) -> p b w", b=nf),
                            in0=pred, in1=wc, op=mybir.AluOpType.is_equal,
                        )
                        nc.vector.tensor_tensor(
                            out=ind_f[:].rearrange("p (b w) -> p b w", b=nf),
                            in0=ind_f[:].rearrange("p (b w) -> p b w", b=nf),
                            in1=nv, op=mybir.AluOpType.mult,
                        )
                    elif nf:
                        nc.vector.memset(ind_f[:, :1], 0.0)
                    ind_t = indp.tile([128, max(nt, 1) * BLK], dt_, tag="indt")
                    if nt and "noind" not in probe:
                        pred = (
                            ramp128[:]
                            .rearrange("p (o w) -> p o w", o=1)
                            .broadcast_to([128, nt, BLK])
                        )
                        wc = (
                            wcol_sb[:, tbc : tbc + nt]
                            .rearrange("p (b o) -> p b o", o=1)
                            .broadcast_to([128, nt, BLK])
                        )
                        nv = (
                            nval_sb[:, tbc : tbc + nt]
                            .rearrange("p (b o) -> p b o", o=1)
                            .broadcast_to([128, nt, BLK])
                        )
                        nc.vector.tensor_tensor(
                            out=ind_t[:].rearrange("p (b w) -> p b w", b=nt),
                            in0=pred, in1=wc, op=mybir.AluOpType.is_equal,
                        )
                        nc.vector.tensor_tensor(
                            out=ind_t[:].rearrange("p (b w) -> p b w", b=nt),
                            in0=ind_t[:].rearrange("p (b w) -> p b w", b=nt),
                            in1=nv, op=mybir.AluOpType.mult,
                        )
                    elif nt:
                        nc.vector.memset(ind_t[:, :1], 0.0)
                    for dep in (*msg_tiles.values(), ind_f, ind_t):
                        if "noabsorb" in probe:
                            break
                        absorb(dep[:, :1])
                    for kk in g["chunks"]:
                        blocks = LS.chunk_blocks[kk]
                        zps = zpsp.tile([128, 128], f32, space="PSUM", tag="z")
                        nc.tensor.matmul(
                            zps[:F, :], lhsT=zero_sb[:, :F], rhs=zero_sb[:, :],
                            start=True, stop=False,
                        )
                        for bi, (h, cs, kind, ric, width, ooff) in enumerate(blocks):
                            if "noblocks" in probe:
                                break
                            last = bi == len(blocks) - 1
                            msg = msg_tiles[h]
                            rhs_tile = ind_f if kind == "full" else ind_t
                            nc.tensor.matmul(
                                zps[:F, ooff : ooff + width],
                                lhsT=msg[:, cs * F : (cs + 1) * F],
                                rhs=rhs_tile[:, ric : ric + width],
                                start=False,
                                stop=last,
                            )
                        z_sb = sbp.tile([F, 128], f32, tag="z_sb")
                        nc.scalar.copy(z_sb[:], zps[:F, :])
                        consume_chunk(kk, z_sb)

            # ---- Layer 1 ----
            def l1_chunk(kk, z_sb):
                absorb(z_sb[:, :1])
                hps = hpsp.tile([H, 128], f32, space="PSUM", tag="h")
                nc.tensor.matmul(hps[:], lhsT=W1_sb[:], rhs=z_sb[:FIN, :], start=True, stop=True)
                h1T = sbp.tile([H, 128], f32, tag="h1T")
                nc.scalar.activation(
                    h1T[:], hps[:], mybir.ActivationFunctionType.Relu, bias=b1_sb[:, :]
                )
                absorb(h1T[:, :1])
                tps = tpsp.tile([128, H], f32, space="PSUM", tag="t")
                nc.tensor.transpose(out=tps[:], in_=h1T[:], identity=ident[:])
                h1n = sbp.tile([128, H], bf16, tag="h1n")
                nc.vector.tensor_copy(out=h1n[:], in_=tps[:])
                nc.sync.dma_start(out=h1sh[kk * 128 : (kk + 1) * 128, :], in_=h1n[:])

            l1_sbs = load_layer_inputs(L1, idx1_t, wcol1_t, nval1_t, "1", f32)
            sparse_layer(
                L1, FIN, l1_sbs,
                xfull[0 : L1.n_lo_rows, :],
                xfull[L1.n_lo_rows : N, :] if L1.n_hi_rows else xfull[0:1, :],
                l1_chunk,
                f32,
            )

            # ---- AllGather h1 ----
            if single_core:
                nc.sync.dma_start(out=h1full[0:PADN, :], in_=h1sh[:, :])
            else:
                nc.gpsimd.collective_compute(
                    "AllGather",
                    mybir.AluOpType.bypass,
                    replica_groups=[list(range(NC))],
                    ins=[h1sh[:, :]],
                    outs=[h1full[:, :]],
                )

            # ---- Layer 2 + head ----
            def l2_chunk(kk, z_sb):
                absorb(z_sb[:, :1])
                h2T_halves = []
                for half_i in range(2):
                    hps = hpsp.tile([H, 128], f32, space="PSUM", tag="h")
                    nc.tensor.matmul(
                        hps[:],
                        lhsT=W2_sb[:, half_i * H : (half_i + 1) * H],
                        rhs=z_sb[:],
                        start=True,
                        stop=True,
                    )
                    h2T = sbp.tile([H, 128], f32, tag=f"h2T{half_i}")
                    nc.scalar.activation(
                        h2T[:],
                        hps[:],
                        mybir.ActivationFunctionType.Relu,
                        bias=b2_sb[:, half_i : half_i + 1],
                    )
                    h2T_halves.append(h2T)
                absorb(h2T_halves[0][:, :1])
                absorb(h2T_halves[1][:, :1])
                qps = qpsp.tile([128, 1], f32, space="PSUM", tag="q")
                for half_i in range(2):
                    nc.tensor.matmul(
                        qps[:],
                        lhsT=h2T_halves[half_i][:],
                        rhs=w3_sb[:, half_i : half_i + 1],
                        start=half_i == 0,
                        stop=half_i == 1,
                    )
                nc.vector.tensor_copy(out=q_sb[:, kk : kk + 1], in_=qps[:])
                nc.tensor.matmul(
                    pool_ps[:],
                    lhsT=Call[:, kk * G : (kk + 1) * G],
                    rhs=q_sb[:, kk : kk + 1],
                    start=kk == 0,
                    stop=kk == CH - 1,
                )

            l2_sbs = load_layer_inputs(L2, idx2_t, wcol2_t, nval2_t, "2", bf16)
            sparse_layer(
                L2, H, l2_sbs,
                h1full[0 : L2.n_lo_rows, :],
                h1full[L2.n_lo_rows : NC * PADN, :] if L2.n_hi_rows else h1full[0:1, :],
                l2_chunk,
                bf16,
            )

            pool_sb = sbp.tile([G, 1], f32, tag="pool")
            nc.vector.tensor_copy(out=pool_sb[:], in_=pool_ps[:])
            nc.sync.dma_start(out=out_t[:, :], in_=pool_sb[:])

    nc.compile()
    return nc


def postprocess(cfg: Cfg, results, host):
    out = np.zeros((cfg.G, 1), dtype=np.float64)
    for r in results:
        out += r["out"].astype(np.float64)
    out += host["c_const"]
    out[host["empty"], 0] = host["linb"]
    return out.astype(np.float32)


from concourse import bass_utils as _bass_utils


def kernel(**inputs) -> np.ndarray:
    cfg = Cfg()
    L1, L2, in_maps, host = preprocess(cfg, inputs)
    nc = build_module(cfg, L1, L2)
    res = _bass_utils.run_bass_kernel_spmd(nc, in_maps, core_ids=list(range(cfg.NC)))
    return postprocess(cfg, res.results, host)
